# revision 36
# baseline (speedup 1.0000x reference)
"""Trainium2 Bass kernel for nn_Net_43052752175597 (2-layer GraphSAGE, aggr=add).

reference:
    A_hat = (A != 0).T with unit diagonal          # [N, N], binary
    h1   = X @ W1;  agg1 = A_hat @ h1 + b1;  x1 = relu(l2norm(agg1))
    h2   = x1 @ W2; agg2 = A_hat @ h2 + b2;  out = l2norm(l2norm(agg2))

Sharding: row-shard A_hat (output nodes) across 8 cores; each core owns 1280
padded nodes (N 10000 -> 10240). Weights replicated.

Default AGG_MODE "drdr" (see _build_nc_drdr): both layers' aggregations
stream the binary A (fp8e4, exact) as the MOVING operand against fp8e4m3
h k-tile PAIRS stationary in DoubleRow perf mode — 0.5 PE cycles per output
column, 2x the plain-fp8 rate — producing the aggregation feature-major in
two 64-feature halves (DoubleRow trades stationary columns for the second
row set). Six PSUM accumulation groups (3 dst chunks x 2 halves) stream
j-pair-major so layer 1 consumes A k-tiles in DMA-arrival order; layer 2
(A resident in SBUF) finishes chunk-major so each chunk's post overlaps the
next chunk's pass. Rel err 1.234e-2 on the harness inputs (fp8e4m3 h
quantization dominates; matches the numpy emulation exactly). Legacy modes
"e3e3" (6.4e-3) / "e3dr" (9.7e-3) are kept in _build_nc_legacy as slower,
higher-precision fallbacks.

Layer-1 post never leaves feature-major and never shifts partitions: biases
are host-repacked per half (b1h), relu+bias is one DVE pass and squares one
ACT pass per 256-col piece, per-node sum-of-squares comes from a ones-vector
matmul accumulated across the halves, and h2 = x1 @ W2 is a split-K pair of
matmuls over the half tiles (w2h host-repacked, SC2 pre-folded so the
l2norm reciprocal is the quantize scale directly). Layer-2 post transposes
back to node-major; the out scale reads the transpose PSUM slots directly
(4 slots >= 4 m-tiles per chunk). l2norm(l2norm(v)) == l2norm(v) up to one
ulp, so the final double-normalize is a single normalize.

The 13 MB/core A slice is DMA'd in 2-k-tile batches on the ACT HWDGE queue
(the Pool queue must stay free for the collectives), k-ordered to match the
aggregation stream. Measured HW A-load is ~15 us (~900 GB/s — the 8 tunneled
cores span multiple chips, each with full HBM-stack bandwidth; see
work/bench_aload.py), so BOTH layers are PE-bound on HW and both use the
phase-A (j-major, arrival order) + phase-B (chunk-major staggered closure)
schedule with posts overlapping later passes. An isolated repeats=33 probe
(work/bench_agg.py) measures one layer's pure aggregation stream at
~16.9 us on HW; 2 x 16.9 = 33.8 us vs the ~36.7 us measured steady state,
i.e. everything outside the two agg streams is fully overlapped and the
kernel sits at the DoubleRow compute floor of this algorithm. AllGather staging is
partition-major so every DMA moves >=512B contiguous runs (sub-512B
transfers pay a 2x read-modify-write penalty). ACT activation tables
(Square/Copy/Sqrt) are pre-warmed at startup so no LoadActFuncSet stalls
the post chains.

PSUM bank map and discipline: 8 explicit [128, 512] bank tiles. B0..B5 hold
the six aggregation groups (half-major: B0..B2 = half-0 chunks, B3..B5 =
half-1); B6/B7 multiplex the h-matmul rotation slots (cols 0:128), the
layer-1 ssq row staging (B7 [0|64, 256:512]) and transposed-ssq columns
(B6 cols 256:266), and the layer-2 transpose slots (cols 0:256). Hardware
semantics: matmul start=True marks the whole bank pending-zero lazily, so
two accumulation groups must never interleave while sharing a bank;
single-shot matmuls may share a bank freely since reads are unaffected by
pending-zero; program order keeps each rowt accumulation pair back-to-back
on PE so no intervening start=True re-marks B7 mid-group.
"""

import sys

sys.path.insert(0, "/opt/trn_rl_repo")

import numpy as np
import ml_dtypes

import concourse.bass as bass  # noqa: F401
import concourse.tile as tile
from concourse import bacc, mybir
from concourse import bass_utils

N = 10000
NP = 10240          # padded node count
F = 256             # input feature dim
H = 128             # hidden dim
N_CORES = 8
PER_CORE = NP // N_CORES        # 1280 nodes per core
M_TILES = PER_CORE // 128       # 10
K_TILES = NP // 128             # 80
MA = 4                          # m-tiles in gather chunk a (sim-swept at
                                # drdr: bigger first chunk feeds more early
                                # k-tiles to the DR layer-1 stream)
MB = M_TILES - MA               # 8
KA = N_CORES * MA               # 16 k-tiles in chunk a
KB = N_CORES * MB               # 64
CHUNKS = [(0, 512), (512, 1024), (1024, 1280)]   # dst column chunks
K_STREAM = K_TILES - 1   # korder[79] = core-7 m9: all padding, skip
ABATCH = 2          # k-tiles per A-load DMA (sim-swept at MA=2)

SC1 = 4.0           # h1 pre-scale (absorbed by l2norm; b1 scaled to match)
SC2 = 64.0          # h2 pre-scale

BF16 = ml_dtypes.bfloat16
E3M4 = ml_dtypes.float8_e3m4
E4M3 = ml_dtypes.float8_e4m3

AGG_MODE = "drdr"

_CACHE = {}


def _h_dt(mode):
    return mybir.dt.float8e4 if mode == "dr" else mybir.dt.float8e3


def _build_nc(agg_mode=None, single_core=False, compile=True, repeats=1,
              fake_ag=False, a_in_loop=False):
    agg_mode = agg_mode or AGG_MODE
    if agg_mode == "drdr":
        return _build_nc_drdr(single_core=single_core, compile=compile,
                              repeats=repeats, fake_ag=fake_ag,
                              a_in_loop=a_in_loop)
    return _build_nc_legacy(agg_mode=agg_mode, single_core=single_core,
                            compile=compile, repeats=repeats, fake_ag=fake_ag)


def _build_nc_drdr(single_core=False, compile=True, repeats=1,
                   fake_ag=False, a_in_loop=False):
    """Full-DoubleRow build: both layers' aggregations stream the binary A
    (fp8e4) as the moving operand with fp8e4 h pairs stationary in DoubleRow
    perf mode (0.5 cyc/row — 2x the plain-fp8 PE rate). j-pair-major streaming
    with SIX simultaneous accumulation groups (3 dst chunks x 2 feature
    halves) so aggregation consumes A k-tiles as they arrive (layer 1) or at
    full PE speed (layer 2, A resident).

    PSUM bank map (8 x [128, 512] fp32 bank tiles, explicit discipline):
      B0..B2  agg half-0 chunks c0..c2 (layer 1 then layer 2 groups)
      B3..B5  agg half-1 chunks c0..c2
      B6      h-mm rotation slot (cols 0:128, pre-agg / L1-post);
              transposed ssq cols 256:266 (L1 post, single-shot writes);
              L2-post transpose slots cols 0:256
      B7      h-mm rotation slot (cols 0:128); ssq row staging
              [0:2, 256:512] (L1 post, partition-alternating); L2-post
              transpose slots cols 0:256
    Single-shot matmuls may share banks with closed/unread-no-more groups
    (reads are unaffected by pending-zero); program order keeps the two
    rowt-accumulation matmuls back-to-back on PE so no intervening
    start=True re-marks B7 mid-group.

    Feature halves (64 partitions) never leave partitions 0:64: biases are
    host-repacked per half (b1h/b2h), and h2 = x1 @ W2 is computed as a
    split-K pair of matmuls over the half tiles (w2h host-repacked), so no
    engine op needs a partition-offset shift.
    """
    fp32 = mybir.dt.float32
    bf16 = mybir.dt.bfloat16
    fp8a = mybir.dt.float8e4   # A tiles: binary, exact in any fp8
    hdt = mybir.dt.float8e4    # h quantized e4m3 (DoubleRow requires e4/e5)
    DR = mybir.MatmulPerfMode.DoubleRow
    Copy = mybir.ActivationFunctionType.Copy
    Square = mybir.ActivationFunctionType.Square
    mult = mybir.AluOpType.mult

    nc = bacc.Bacc(
        "TRN2",
        target_bir_lowering=False,
        debug=False,
        enable_asserts=True,
        num_devices=1 if single_core else N_CORES,
    )

    a_pre = nc.dram_tensor("a_pre", [K_TILES, 128, PER_CORE], fp8a,
                           kind="ExternalInput").ap()
    xt = nc.dram_tensor("xt", [128, 2, PER_CORE], bf16,
                        kind="ExternalInput").ap()
    w1 = nc.dram_tensor("w1", [128, 2, H], bf16, kind="ExternalInput").ap()
    w2h = nc.dram_tensor("w2h", [64, 2, H], bf16, kind="ExternalInput").ap()
    b1h = nc.dram_tensor("b1h", [64, 2], fp32, kind="ExternalInput").ap()
    b2h = nc.dram_tensor("b2h", [64, 2], fp32, kind="ExternalInput").ap()
    ones = nc.dram_tensor("ones", [128, 1], bf16, kind="ExternalInput").ap()
    ident = nc.dram_tensor("ident", [128, 128], fp32, kind="ExternalInput").ap()
    out = nc.dram_tensor("out", [PER_CORE, H], fp32, kind="ExternalOutput").ap()

    n_sp = K_TILES // 2

    with tile.TileContext(nc) as tc:
        with tc.tile_pool(name="const", bufs=1) as cpool, \
             tc.tile_pool(name="acache", bufs=1) as apool, \
             tc.tile_pool(name="hfull", bufs=1) as hpool, \
             tc.tile_pool(name="work", bufs=1) as wpool, \
             tc.tile_pool(name="psum", bufs=1, space="PSUM") as ppool, \
             tc.tile_pool(name="dram", bufs=2, space="DRAM") as dpool:

            # ---- constants into SBUF ----
            t_xt = cpool.tile([128, 2, PER_CORE], bf16)
            t_w1 = cpool.tile([128, 2, H], bf16)
            t_w2h = cpool.tile([64, 2, H], bf16)
            t_b1h = cpool.tile([64, 2], fp32)
            t_b2h = cpool.tile([64, 2], fp32)
            t_ones = cpool.tile([128, 1], bf16)
            t_id = cpool.tile([128, 128], fp32)
            nc.sync.dma_start(t_w1[:], w1[:])
            nc.sync.dma_start(t_xt[:, :, 0:MA * 128], xt[:, :, 0:MA * 128])
            nc.sync.dma_start(t_xt[:, :, MA * 128:], xt[:, :, MA * 128:])
            # ident + w2h are needed only at layer-1 post; they trail the
            # xt loads on the sync queue so the A feed (ACT queue) is never
            # delayed by them.
            nc.sync.dma_start(t_id[:], ident[:])
            nc.sync.dma_start(t_w2h[:], w2h[:])

            # small consts first (tiny, and the ACT warm ops below need
            # t_ones early so the ACT queue never blocks on late data)
            nc.scalar.dma_start(t_b1h[:], b1h[:])
            nc.scalar.dma_start(t_b2h[:], b2h[:])
            nc.scalar.dma_start(t_ones[:], ones[:])

            # pre-warm the ACT piecewise-poly tables (Square/Copy/Sqrt):
            # the auto-inserted LoadActFuncSet instructions land here at
            # startup instead of stalling the layer-1 post critical chain.
            t_warm = wpool.tile([1, 4], fp32)
            nc.scalar.activation(t_warm[:, 0:1], t_ones[0:1, 0:1], Square)
            nc.scalar.sqrt(t_warm[:, 1:2], t_warm[:, 0:1])
            nc.scalar.activation(t_warm[:, 2:3], t_warm[:, 0:1], Copy)

            # whole per-core A slice, k-ordered, batched on the ACT HWDGE
            # queue (Pool queue must stay free for the collectives).
            t_a = apool.tile([128, K_TILES, PER_CORE], fp8a)

            def load_a():
                for j0 in range(0, K_TILES, ABATCH):
                    nc.scalar.dma_start(
                        t_a[:, j0:j0 + ABATCH, :],
                        a_pre[j0:j0 + ABATCH].rearrange("j p n -> p j n"))

            if not a_in_loop:
                load_a()

            # gathered features
            t_h1a = hpool.tile([128, KA, H], hdt)
            t_h1b = hpool.tile([128, KB, H], hdt)
            t_h2a = hpool.tile([128, KA, H], hdt)
            t_h2b = hpool.tile([128, KB, H], hdt)

            # working tiles
            t_h1s = wpool.tile([128, M_TILES, H], hdt)    # own-shard h1
            t_h2s = wpool.tile([128, M_TILES, H], hdt)
            t_x1Th = wpool.tile([64, 2, PER_CORE], bf16)  # relu halves, f-major
            t_sqTh = wpool.tile([64, 2, PER_CORE], bf16)  # square halves
            t_row = wpool.tile([1, PER_CORE], fp32)       # ssq row staging
            t_nrm = wpool.tile([128, M_TILES], fp32)
            t_inv = wpool.tile([128, M_TILES], fp32)
            t_sqs = wpool.tile([128, H], bf16)            # Square scratch
            t_ssqN = wpool.tile([128, M_TILES], fp32)     # L2 ssq accum
            t_aggTh0 = wpool.tile([64, PER_CORE], fp32)   # L2 staging halves
            t_aggTh1 = wpool.tile([64, PER_CORE], fp32)
            t_aggTh = [t_aggTh0, t_aggTh1]
            t_outf = wpool.tile([128, M_TILES, H], fp32)

            # PSUM: 8 explicit bank tiles (see docstring for the map)
            B = [ppool.tile([128, 512], fp32, name=f"bank{i}")
                 for i in range(8)]

            def agg_bank(half, ci):
                return B[3 * half + ci]

            def mm_slot(i):
                # h1 rotation (pre-agg): B6, B7, B0, B1 — B0/B1's layer-1 agg
                # groups start only after these slots' reads complete.
                return [B[6], B[7], B[0], B[1]][i % 4][:, 0:H]

            def mm_slot2(ci, i):
                # h2 rotation during L1 post, per chunk: only banks whose
                # layer-1 agg data has already been consumed (B6/B7 + the
                # chunk's own two agg banks).
                return [B[6], B[7], B[ci], B[3 + ci]][i % 4][:, 0:H]

            def ssq_cols(m0, m1):
                return B[6][:, 256 + m0:256 + m1]

            def rowt(piece):
                p = 64 * (piece % 2)
                return B[7][p:p + 1, 256:512]

            def tr_slot(i):
                return [B[6], B[7]][i % 2][:, 128 * ((i // 2) % 2):
                                           128 * ((i // 2) % 2) + 128]

            def gather(t_hs, part, t_dst, rep):
                """AllGather chunk ('a': m 0:MA | 'b': m MA:10) into t_dst.

                Staging is PARTITION-major ([128, mm*H]) so both the SBUF->
                DRAM stage and the gathered load move mm*H-byte contiguous
                runs per (core, partition) — >=512B for part b, dodging the
                sub-512B read-modify-write DMA penalty that a [node, feat]
                layout pays (128B runs). The gathered DRAM buffer is then
                [(c p), (m f)]; the load rearrange keeps t_dst's c-major
                k-tile order (korder unchanged)."""
                m0, m1 = (0, MA) if part == "a" else (MA, M_TILES)
                mm = m1 - m0
                kk = N_CORES * mm
                sh = dpool.tile([128, mm * H], hdt, tag=f"sh_{part}",
                                name=f"sh_{part}_{rep}")
                g = dpool.tile([N_CORES * 128, mm * H], hdt, tag=f"g_{part}",
                               addr_space="Shared", name=f"g_{part}_{rep}")
                nc.sync.dma_start(sh[:], t_hs[:, m0:m1, :])
                if single_core or fake_ag:
                    for c in range(N_CORES):
                        nc.sync.dma_start(
                            t_dst[:, c * mm:(c + 1) * mm, :], sh[:])
                else:
                    nc.gpsimd.collective_compute(
                        "AllGather", mybir.AluOpType.bypass,
                        replica_groups=[list(range(N_CORES))],
                        ins=[sh.opt()], outs=[g.opt()],
                    )
                    gr = g[:].rearrange("(c p) (m f) -> p c m f", p=128, f=H)
                    dv = t_dst[:].rearrange("p (c m) f -> p c m f", m=mm)
                    ch = N_CORES // 2
                    nc.sync.dma_start(dv[:, 0:ch], gr[:, 0:ch])
                    nc.sync.dma_start(dv[:, ch:], gr[:, ch:])

            def inv_chain(ssq_src, mr):
                nc.scalar.sqrt(t_nrm[:, mr], ssq_src(mr))
                nc.vector.tensor_scalar_max(t_nrm[:, mr], t_nrm[:, mr], 1e-12)
                nc.vector.reciprocal(t_inv[:, mr], t_nrm[:, mr])

            def out_m(m, src):
                if m % 2 == 0:
                    nc.scalar.activation(t_outf[:, m, :], src, Copy,
                                         scale=t_inv[:, m:m + 1])
                else:
                    nc.vector.tensor_scalar_mul(t_outf[:, m, :], src,
                                                t_inv[:, m:m + 1])

            for _rep in range(repeats):
                if a_in_loop:
                    load_a()   # benchmarking: charge the A DMA to every rep
                # ====== h1 = X@W1 (bf16), quantize e4m3, chunked gathers
                for m in range(M_TILES):
                    ps = mm_slot(m)
                    for k in range(2):
                        nc.tensor.matmul(ps, t_xt[:, k, m * 128:(m + 1) * 128],
                                         t_w1[:, k, :],
                                         start=(k == 0), stop=(k == 1))
                    nc.vector.tensor_scalar_mul(t_h1s[:, m, :], ps, SC1)
                    if m == MA - 1:
                        gather(t_h1s, "a", t_h1a, _rep)
                    elif m == M_TILES - 1:
                        gather(t_h1s, "b", t_h1b, _rep)

                # PE p-state warm-up: the tensor engine ramps 0.65 -> 1.2 ->
                # 2.4 GHz over ~3us of CONTINUOUS execution and decays when
                # idle. Dependency-free filler matmuls bridge the AG1a
                # flight (PE would otherwise idle) so the layer-1 stream
                # starts at full clock. Single-shot writes to the B6/B7
                # h-mm slot region (same discipline as the h1 matmuls);
                # bounded cost if the gather lands early, ~1.5-2us ramp
                # saving otherwise.
                for wi in range(12):
                    nc.tensor.matmul(
                        [B[6], B[7]][wi % 2][:, 0:H],
                        t_w1[:, 0, :], t_w1[:, 1, :],
                        start=True, stop=True)

                # ====== Layer 1 aggregation. HW loads the 13MB A slice at
                # ~900 GB/s (multi-chip HBM; measured with work/
                # bench_aload.py), so layer 1 is PE-bound, not feed-bound.
                # Phase A streams j-major in gather/A-arrival order (a tiles
                # + the first b half-load); phase B completes each of the
                # six groups chunk-major over the last b half so closures
                # stagger and the per-piece posts overlap later passes.
                SPB1 = (KA + KB // 2) // 2
                for sp in range(SPB1):
                    j = 2 * sp
                    hsrc = (t_h1a[:, j:j + 2, :] if j < KA
                            else t_h1b[:, j - KA:j - KA + 2, :])
                    for half in range(2):
                        lhsT = hsrc[:, :, 64 * half:64 * half + 64]
                        for ci, (c0, c1) in enumerate(CHUNKS):
                            nc.tensor.matmul(
                                agg_bank(half, ci)[0:64, 0:c1 - c0], lhsT,
                                t_a[:, j:j + 2, c0:c1], perf_mode=DR,
                                start=(sp == 0), stop=False)

                def l1_passB(ci, half):
                    c0, c1 = CHUNKS[ci]
                    for sp in range(SPB1, n_sp):
                        j = 2 * sp
                        hsrc = t_h1b[:, j - KA:j - KA + 2, :]
                        nc.tensor.matmul(
                            agg_bank(half, ci)[0:64, 0:c1 - c0],
                            hsrc[:, :, 64 * half:64 * half + 64],
                            t_a[:, j:j + 2, c0:c1], perf_mode=DR,
                            start=False, stop=(sp == n_sp - 1))

                # ====== Layer 1 post (feature-major halves) + h2 + gathers.
                # Per-256-col-piece (2 m-tiles): each piece's relu/square ->
                # ssq -> inv -> h2 -> quantize chain runs as soon as its
                # chunk's two groups close, overlapping later phase-B passes.
                def l1_piece(pi):
                    p0, p1 = 256 * pi, 256 * pi + 256
                    ci = p0 // 512
                    b0 = p0 - CHUNKS[ci][0]
                    for half in range(2):
                        src = agg_bank(half, ci)[0:64, b0:b0 + 256]
                        nc.vector.tensor_scalar(
                            t_x1Th[:, half, p0:p1], src,
                            t_b1h[:, half:half + 1], 0.0,
                            op0=mybir.AluOpType.add, op1=mybir.AluOpType.max)
                        nc.scalar.activation(t_sqTh[:, half, p0:p1], src,
                                             Square,
                                             bias=t_b1h[:, half:half + 1])
                    # per-node sum-of-squares: ones^T @ squares, both halves
                    # accumulated; transpose to node-major psum cols
                    rt = rowt(pi)[:, 0:256]
                    for half in range(2):
                        nc.tensor.matmul(rt, t_ones[0:64, :],
                                         t_sqTh[:, half, p0:p1],
                                         start=(half == 0), stop=(half == 1))
                    nc.vector.tensor_copy(t_row[:, p0:p1], rt)
                    for m in range(p0 // 128, p0 // 128 + 2):
                        nc.tensor.matmul(
                            ssq_cols(m, m + 1),
                            t_row[:, m * 128:(m + 1) * 128],
                            t_id[0:1, 0:1],
                            is_transpose=True, start=True, stop=True)
                    mr = slice(p0 // 128, p0 // 128 + 2)
                    inv_chain(lambda mr: ssq_cols(mr.start, mr.stop), mr)
                    # SC2 is folded into w2h host-side, so the l2norm scale
                    # t_inv is the quantize scale directly (no scl pass).
                    for m in range(p0 // 128, p0 // 128 + 2):
                        ps = mm_slot2(ci, m)
                        for half in range(2):
                            nc.tensor.matmul(
                                ps, t_x1Th[:, half, m * 128:(m + 1) * 128],
                                t_w2h[:, half, :],
                                start=(half == 0), stop=(half == 1))
                        if m % 2 == 0:
                            nc.scalar.activation(t_h2s[:, m, :], ps, Copy,
                                                 scale=t_inv[:, m:m + 1])
                        else:
                            nc.vector.tensor_scalar_mul(t_h2s[:, m, :], ps,
                                                        t_inv[:, m:m + 1])
                        if m == MA - 1:
                            gather(t_h2s, "a", t_h2a, _rep)
                        elif m == M_TILES - 1:
                            gather(t_h2s, "b", t_h2b, _rep)

                l1_passB(0, 0)
                l1_passB(0, 1)
                l1_passB(1, 0)
                l1_piece(0)
                l1_piece(1)
                l1_passB(1, 1)
                l1_passB(2, 0)
                l1_piece(2)
                l1_piece(3)
                l1_passB(2, 1)
                l1_piece(4)

                # ====== Layer 2: chunk-major DoubleRow at full PE speed
                # (A resident in SBUF), software-pipelined so chunk ci's
                # post (transpose to node-major, ssq via Square-accum,
                # l2norm, out) runs while chunk ci+1 streams. Emission
                # order interleaves post(ci)'s PE transposes after the
                # NEXT chunk's half-0 pass so PE never stalls on the DVE
                # staging copies.
                # phase A: all six groups stream j-major in gather-arrival
                # order (a tiles, then the first b half-load) — overlaps the
                # AG2b flight and the layer-1 post tail.
                SP_B = KA // 2
                for sp in range(SP_B):
                    j = 2 * sp
                    hsrc = (t_h2a[:, j:j + 2, :] if j < KA
                            else t_h2b[:, j - KA:j - KA + 2, :])
                    for half in range(2):
                        lhsT = hsrc[:, :, 64 * half:64 * half + 64]
                        for ci, (c0, c1) in enumerate(CHUNKS):
                            nc.tensor.matmul(
                                agg_bank(half, ci)[0:64, 0:c1 - c0], lhsT,
                                t_a[:, j:j + 2, c0:c1], perf_mode=DR,
                                start=(sp == 0), stop=False)

                def l2_pass(ci, half):
                    # phase B: per-group completion over the last b half
                    # (chunk-major so posts pipeline against later groups'
                    # passes)
                    c0, c1 = CHUNKS[ci]
                    for sp in range(SP_B, n_sp):
                        j = 2 * sp
                        hsrc = t_h2b[:, j - KA:j - KA + 2, :]
                        nc.tensor.matmul(
                            agg_bank(half, ci)[0:64, 0:c1 - c0],
                            hsrc[:, :, 64 * half:64 * half + 64],
                            t_a[:, j:j + 2, c0:c1], perf_mode=DR,
                            start=False, stop=(sp == n_sp - 1))
                    # stage to SBUF with the bias folded in (frees the bank
                    # and feeds the node-major transposes)
                    nc.vector.tensor_scalar_add(
                        t_aggTh[half][:, c0:c1],
                        agg_bank(half, ci)[0:64, 0:c1 - c0],
                        t_b2h[:, half:half + 1])

                def l2_post(ci):
                    c0, c1 = CHUNKS[ci]
                    for m in range(c0 // 128, c1 // 128):
                        pst = tr_slot(m)
                        for half in range(2):
                            nc.tensor.transpose(
                                pst[:, 64 * half:64 * half + 64],
                                t_aggTh[half][:, m * 128:(m + 1) * 128],
                                t_id[0:64, 0:64])
                        nc.scalar.activation(t_sqs[:], pst, Square,
                                             accum_out=t_ssqN[:, m:m + 1])
                    # single normalize: l2norm(l2norm(v)) == l2norm(v) up to
                    # one-ulp rounding of the first normalize (~1e-7), far
                    # below the fp8 noise floor — skip the composite chain.
                    mr = slice(c0 // 128, c1 // 128)
                    inv_chain(lambda mr: t_ssqN[:, mr], mr)
                    for m in range(c0 // 128, c1 // 128):
                        out_m(m, tr_slot(m))
                    nc.sync.dma_start(
                        out[:].rearrange(
                            "(mm p) f -> p mm f",
                            p=128)[:, c0 // 128:c1 // 128, :],
                        t_outf[:, c0 // 128:c1 // 128, :])

                l2_pass(0, 0)
                l2_pass(0, 1)
                l2_pass(1, 0)
                l2_post(0)
                l2_pass(1, 1)
                l2_pass(2, 0)
                l2_post(1)
                l2_pass(2, 1)
                l2_post(2)

    if compile:
        nc.compile()
    return nc


def _build_nc_legacy(agg_mode=None, single_core=False, compile=True,
                     repeats=1, fake_ag=False):
    """Build + compile the 8-core SPMD Bass kernel. Returns the Bacc object.

    single_core=True builds a 1-core variant with collectives replaced by
    equivalent-byte local DMAs — only for TimelineSim profiling.
    fake_ag=True keeps 8 cores but fakes the collectives the same way
    (WRONG results — collective-cost measurement only).
    repeats>1 re-runs the whole 2-layer body (benchmarking only).
    """
    agg_mode = agg_mode or AGG_MODE
    mode1, mode2 = agg_mode[:2], agg_mode[2:]
    assert mode1 == "e3" and mode2 in ("e3", "dr")
    fp32 = mybir.dt.float32
    bf16 = mybir.dt.bfloat16
    fp8a = mybir.dt.float8e4   # A tiles: binary, exact in any fp8
    dt1, dt2 = _h_dt(mode1), _h_dt(mode2)
    DR = mybir.MatmulPerfMode.DoubleRow
    Copy = mybir.ActivationFunctionType.Copy
    Relu = mybir.ActivationFunctionType.Relu
    Square = mybir.ActivationFunctionType.Square
    mult = mybir.AluOpType.mult

    nc = bacc.Bacc(
        "TRN2",
        target_bir_lowering=False,
        debug=False,
        enable_asserts=True,
        num_devices=1 if single_core else N_CORES,
    )

    a_pre = nc.dram_tensor("a_pre", [K_TILES, 128, PER_CORE], fp8a,
                           kind="ExternalInput").ap()
    xt = nc.dram_tensor("xt", [128, 2, PER_CORE], bf16,
                        kind="ExternalInput").ap()
    w1 = nc.dram_tensor("w1", [128, 2, H], bf16, kind="ExternalInput").ap()
    w2 = nc.dram_tensor("w2", [128, H], bf16, kind="ExternalInput").ap()
    b1c = nc.dram_tensor("b1c", [128, 1], fp32, kind="ExternalInput").ap()
    b2c = nc.dram_tensor("b2c", [128, 1], fp32, kind="ExternalInput").ap()
    b2h = nc.dram_tensor("b2h", [64, 2], fp32, kind="ExternalInput").ap()
    ones = nc.dram_tensor("ones", [128, 1], bf16, kind="ExternalInput").ap()
    ident = nc.dram_tensor("ident", [128, 128], fp32, kind="ExternalInput").ap()
    out = nc.dram_tensor("out", [PER_CORE, H], fp32, kind="ExternalOutput").ap()

    with tile.TileContext(nc) as tc:
        with tc.tile_pool(name="const", bufs=1) as cpool, \
             tc.tile_pool(name="acache", bufs=1) as apool, \
             tc.tile_pool(name="hfull", bufs=1) as hpool, \
             tc.tile_pool(name="work", bufs=1) as wpool, \
             tc.tile_pool(name="psum_agg", bufs=1, space="PSUM") as pagg, \
             tc.tile_pool(name="psum_mm", bufs=1, space="PSUM") as pmm, \
             tc.tile_pool(name="psum_ssq", bufs=1, space="PSUM") as pssq, \
             tc.tile_pool(name="psum_tr", bufs=1, space="PSUM") as ptr, \
             tc.tile_pool(name="dram", bufs=2, space="DRAM") as dpool:

            # ---- constants into SBUF ----
            t_xt = cpool.tile([128, 2, PER_CORE], bf16)
            t_w1 = cpool.tile([128, 2, H], bf16)
            t_w2 = cpool.tile([128, H], bf16)
            t_b1c = cpool.tile([128, 1], fp32)
            t_b2c = cpool.tile([128, 1], fp32)
            t_b2h = cpool.tile([64, 2], fp32)
            t_ones = cpool.tile([128, 1], bf16)
            t_id = cpool.tile([128, 128], fp32)
            nc.sync.dma_start(t_xt[:, :, 0:MA * 128], xt[:, :, 0:MA * 128])
            nc.sync.dma_start(t_xt[:, :, MA * 128:], xt[:, :, MA * 128:])
            nc.sync.dma_start(t_w1[:], w1[:])
            nc.sync.dma_start(t_w2[:], w2[:])

            # whole per-core A slice, k-ordered (chunk-a k's first), batched
            # loads on the ACT HWDGE queue (idle until layer-1 post) so the
            # Pool queue stays free for the collectives — otherwise AG1
            # cannot even issue until ~28us of serial A descriptor generation
            # completes. Layer-1 agg consumes k-tiles as they arrive. The
            # small constants (not needed until layer-1 post) load last.
            t_a = apool.tile([128, K_TILES, PER_CORE], fp8a)
            for j0 in range(0, K_TILES, ABATCH):
                nc.scalar.dma_start(
                    t_a[:, j0:j0 + ABATCH, :],
                    a_pre[j0:j0 + ABATCH].rearrange("j p n -> p j n"))
            nc.scalar.dma_start(t_b1c[:], b1c[:])
            nc.scalar.dma_start(t_b2c[:], b2c[:])
            nc.scalar.dma_start(t_b2h[:], b2h[:])
            nc.scalar.dma_start(t_ones[:], ones[:])
            nc.scalar.dma_start(t_id[:], ident[:])

            # gathered features (per layer dtype)
            t_h1a = hpool.tile([128, KA, H], dt1)
            t_h1b = hpool.tile([128, KB, H], dt1)
            t_h2a = hpool.tile([128, KA, H], dt2)
            t_h2b = hpool.tile([128, KB, H], dt2)

            # working tiles
            t_h1s = wpool.tile([128, M_TILES, H], dt1)    # own-shard h1
            t_h2s = wpool.tile([128, M_TILES, H], dt2)
            t_x1T = wpool.tile([128, PER_CORE], bf16)     # relu(agg1'), f-major
            t_sqT = wpool.tile([128, PER_CORE], bf16)     # squares, f-major
            t_row = wpool.tile([1, PER_CORE], fp32)       # ssq row staging
            t_ssqN = wpool.tile([128, M_TILES], fp32)     # dr-mode ssq accum
            t_nrm = wpool.tile([128, M_TILES], fp32)
            t_inv = wpool.tile([128, M_TILES], fp32)
            t_n2 = wpool.tile([128, M_TILES], fp32)
            t_inv2 = wpool.tile([128, M_TILES], fp32)
            t_scl = wpool.tile([128, M_TILES], fp32)
            t_sqs = wpool.tile([128, H], bf16)            # Square scratch (dr)
            t_aggN = wpool.tile([128, M_TILES, H], fp32)  # L2 node-major agg
            t_outf = wpool.tile([128, M_TILES, H], fp32)
            t_aggT = wpool.tile([128, PER_CORE], fp32)    # L2-e3 staging
            t_aggTh0 = wpool.tile([64, PER_CORE], fp32)   # L2-dr staging
            t_aggTh1 = wpool.tile([64, PER_CORE], fp32)
            t_aggTh = [t_aggTh0, t_aggTh1]

            # PSUM banks (see module docstring for sharing rules). The tile
            # framework serializes same-tile accumulation-group starts after
            # all prior reads of that tile, so latency-critical rotations
            # (h-matmuls, L2 transposes) alternate between TWO banks.
            t_ps_mm0 = pmm.tile([128, 512], fp32, name="t_ps_mm0")
            t_ps_mm1 = pmm.tile([128, 512], fp32, name="t_ps_mm1")
            _mm_banks = [t_ps_mm0, t_ps_mm1]
            t_ps_rowt = pssq.tile([1, 512], fp32, name="t_ps_rowt")
            t_ps_tr0 = ptr.tile([128, 512], fp32, name="t_ps_tr0")
            t_ps_tr1 = ptr.tile([128, 512], fp32, name="t_ps_tr1")
            _tr_banks = [t_ps_tr0, t_ps_tr1]

            # h1/h2 matmul slots rotate over FOUR banks (borrowing the
            # transpose banks, idle during the h phases; they only write
            # cols 0:128, never touching tr0's ssq columns 256+).
            def mm_slot(i):
                return (_mm_banks + _tr_banks)[i % 4][:, 0:H]

            def tr_slot(i):
                return _tr_banks[i % 2][:, 128 * ((i // 2) % 2):
                                        128 * ((i // 2) % 2) + 128]

            # transposed per-node ssq lives in tr-bank-0's spare columns
            # (256:266); all writers there are single-shot transposes.
            def ssq_cols(m0, m1):
                return t_ps_tr0[:, 256 + m0:256 + m1]

            def gather(t_hs, part, t_dst, h_dt, rep):
                """AllGather chunk ('a': m 0:6 | 'b': m 6:10) into t_dst."""
                m0, m1 = (0, MA) if part == "a" else (MA, M_TILES)
                rows = (m1 - m0) * 128
                kk = N_CORES * (m1 - m0)
                sh = dpool.tile([rows, H], h_dt, tag=f"sh_{part}",
                                name=f"sh_{part}_{rep}")
                g = dpool.tile([kk * 128, H], h_dt, tag=f"g_{part}",
                               addr_space="Shared", name=f"g_{part}_{rep}")
                nc.sync.dma_start(
                    sh[:].rearrange("(m p) f -> p m f", p=128),
                    t_hs[:, m0:m1, :])
                if single_core or fake_ag:
                    for c in range(N_CORES):
                        nc.sync.dma_start(
                            t_dst[:, c * (m1 - m0):(c + 1) * (m1 - m0), :],
                            sh[:].rearrange("(m p) f -> p m f", p=128))
                else:
                    nc.gpsimd.collective_compute(
                        "AllGather", mybir.AluOpType.bypass,
                        replica_groups=[list(range(N_CORES))],
                        ins=[sh.opt()], outs=[g.opt()],
                    )
                    # split the SBUF load so aggregation starts on the first
                    # half of the gathered k-tiles
                    kh = kk // 2
                    gr = g[:].rearrange("(k p) f -> p k f", p=128)
                    nc.sync.dma_start(t_dst[:, 0:kh, :], gr[:, 0:kh, :])
                    nc.sync.dma_start(t_dst[:, kh:kk, :], gr[:, kh:kk, :])

            def h_of(t_pa, t_pb, j):
                return t_pa[:, j, :] if j < KA else t_pb[:, j - KA, :]

            def ssq_mm(ci, sq_src):
                """ones^T @ squares chunk -> ssq row psum; stage + transpose
                into the [128, M_TILES] node-major psum tile."""
                c0, c1 = CHUNKS[ci]
                nc.tensor.matmul(t_ps_rowt[:, 0:c1 - c0], t_ones[:],
                                 sq_src[:, c0:c1], start=True, stop=True)
                nc.vector.tensor_copy(t_row[:, c0:c1], t_ps_rowt[:, 0:c1 - c0])
                for m in range(c0 // 128, c1 // 128):
                    nc.tensor.matmul(
                        ssq_cols(m, m + 1),
                        t_row[:, m * 128:(m + 1) * 128], t_id[0:1, 0:1],
                        is_transpose=True, start=True, stop=True)

            def inv_chain(ssq_src, mr, with_l2l2=False):
                """t_inv[:, mr] = 1/max(sqrt(ssq), eps); optionally the
                double-l2norm composite scale. ssq_src: callable mr->AP."""
                nc.scalar.sqrt(t_nrm[:, mr], ssq_src(mr))
                nc.vector.tensor_scalar_max(t_nrm[:, mr], t_nrm[:, mr], 1e-12)
                nc.vector.reciprocal(t_inv[:, mr], t_nrm[:, mr])
                if with_l2l2:
                    # out = l2norm(l2norm(agg)): ||agg*inv|| = nrm*inv
                    nc.vector.tensor_tensor(t_n2[:, mr], t_nrm[:, mr],
                                            t_inv[:, mr], op=mult)
                    nc.vector.tensor_scalar_max(t_n2[:, mr], t_n2[:, mr],
                                                1e-12)
                    nc.vector.reciprocal(t_inv2[:, mr], t_n2[:, mr])
                    nc.vector.tensor_tensor(t_inv[:, mr], t_inv[:, mr],
                                            t_inv2[:, mr], op=mult)

            def out_m(m, src, dma=True):
                if m % 2 == 0:
                    nc.scalar.activation(t_outf[:, m, :], src, Copy,
                                         scale=t_inv[:, m:m + 1])
                else:
                    nc.vector.tensor_scalar_mul(t_outf[:, m, :], src,
                                                t_inv[:, m:m + 1])
                if dma:
                    nc.sync.dma_start(
                        out[:].rearrange("(mm p) f -> p mm f", p=128)[:, m, :],
                        t_outf[:, m, :])

            for _rep in range(repeats):
                # ====== Layer 1: h1 = X@W1 (bf16), quantize, chunked gather
                for m in range(M_TILES):
                    ps = mm_slot(m)
                    for k in range(2):
                        nc.tensor.matmul(ps, t_xt[:, k, m * 128:(m + 1) * 128],
                                         t_w1[:, k, :],
                                         start=(k == 0), stop=(k == 1))
                    nc.vector.tensor_scalar_mul(t_h1s[:, m, :], ps, SC1)
                    if m == MA - 1:
                        gather(t_h1s, "a", t_h1a, dt1, _rep)
                    elif m == M_TILES - 1:
                        gather(t_h1s, "b", t_h1b, dt1, _rep)

                # ====== Layer 1 aggregation: j-major (A arrival order),
                # 3 interleaved chunk groups on separate banks.
                af = [pagg.tile([128, 512], fp32, tag=f"af{ci}",
                                name=f"af{ci}_l1r{_rep}") for ci in range(3)]
                for j in range(K_STREAM):
                    hap = h_of(t_h1a, t_h1b, j)
                    for ci, (c0, c1) in enumerate(CHUNKS):
                        nc.tensor.matmul(af[ci][:, 0:c1 - c0], hap,
                                         t_a[:, j, c0:c1],
                                         start=(j == 0),
                                         stop=(j == K_STREAM - 1))

                # ====== Layer 1 post, all in feature-major:
                # x1^T = relu(agg'+b); squares -> ssq via ones-matmul;
                # h2 = x1@W2 with the l2norm scale folded into the quantize.
                # per-chunk pipeline so AG2-a (which needs only h2 m0:MA,
                # inside chunk c0) launches without waiting for chunks c1/c2's
                # squares + ssq chain.
                for ci, (c0, c1) in enumerate(CHUNKS):
                    src = af[ci][:, 0:c1 - c0]
                    nc.vector.tensor_scalar(t_x1T[:, c0:c1], src, t_b1c[:],
                                            0.0, op0=mybir.AluOpType.add,
                                            op1=mybir.AluOpType.max)
                    nc.scalar.activation(t_sqT[:, c0:c1], src, Square,
                                         bias=t_b1c[:])
                    ssq_mm(ci, t_sqT)
                    mr = slice(c0 // 128, c1 // 128)
                    inv_chain(lambda mr: ssq_cols(mr.start, mr.stop), mr)
                    nc.vector.tensor_scalar_mul(t_scl[:, mr], t_inv[:, mr],
                                                SC2)
                    for m in range(c0 // 128, c1 // 128):
                        ps = mm_slot(m)
                        nc.tensor.matmul(ps, t_x1T[:, m * 128:(m + 1) * 128],
                                         t_w2[:], start=True, stop=True)
                        if m % 2 == 0:
                            nc.scalar.activation(t_h2s[:, m, :], ps, Copy,
                                                 scale=t_scl[:, m:m + 1])
                        else:
                            nc.vector.tensor_scalar_mul(t_h2s[:, m, :], ps,
                                                        t_scl[:, m:m + 1])
                        if m == MA - 1:
                            gather(t_h2s, "a", t_h2a, dt2, _rep)
                        elif m == M_TILES - 1:
                            gather(t_h2s, "b", t_h2b, dt2, _rep)

                # ====== Layer 2 aggregation + post ======
                af2 = [pagg.tile([128, 512], fp32, tag=f"af{ci}",
                                 name=f"af{ci}_l2r{_rep}") for ci in range(3)]
                if mode2 == "e3":
                    # chunk-major passes so each chunk's post overlaps the
                    # next pass (bank reuse across passes is safe: the groups
                    # never interleave).
                    def l2_pass(ci):
                        c0, c1 = CHUNKS[ci]
                        for j in range(K_STREAM):
                            nc.tensor.matmul(
                                af2[ci][:, 0:c1 - c0],
                                h_of(t_h2a, t_h2b, j), t_a[:, j, c0:c1],
                                start=(j == 0), stop=(j == K_STREAM - 1))

                    def l2_post(ci):
                        c0, c1 = CHUNKS[ci]
                        src = af2[ci][:, 0:c1 - c0]
                        nc.vector.tensor_scalar_add(t_aggT[:, c0:c1], src,
                                                    t_b2c[:])
                        nc.scalar.activation(t_sqT[:, c0:c1], src, Square,
                                             bias=t_b2c[:])
                        ssq_mm(ci, t_sqT)
                        mr = slice(c0 // 128, c1 // 128)
                        inv_chain(lambda mr: ssq_cols(mr.start, mr.stop), mr,
                                  with_l2l2=True)
                        for m in range(c0 // 128, c1 // 128):
                            pst = tr_slot(m)
                            nc.tensor.transpose(
                                pst, t_aggT[:, m * 128:(m + 1) * 128],
                                t_id[:])
                            out_m(m, pst, dma=False)
                        nc.sync.dma_start(
                            out[:].rearrange(
                                "(mm p) f -> p mm f",
                                p=128)[:, c0 // 128:c1 // 128, :],
                            t_outf[:, c0 // 128:c1 // 128, :])

                    # post(ci) needs only pass(ci); running post(1) before
                    # pass(2) moves its PE work off the critical tail, which
                    # then contains just post(2).
                    l2_pass(0)
                    l2_pass(1)
                    l2_post(0)
                    l2_post(1)
                    l2_pass(2)
                    l2_post(2)
                else:
                    # DoubleRow, half-major so the three chunk groups reuse
                    # the af banks across halves (half-1 starts only after
                    # half-0's staging copies read them).
                    n_sp = K_TILES // 2
                    for half in range(2):
                        for sp in range(n_sp):
                            j = 2 * sp
                            hsrc = (t_h2a[:, j:j + 2, :] if j < KA
                                    else t_h2b[:, j - KA:j - KA + 2, :])
                            lhsT = hsrc[:, :, 64 * half:64 * half + 64]
                            for ci, (c0, c1) in enumerate(CHUNKS):
                                nc.tensor.matmul(
                                    af2[ci][0:64, 0:c1 - c0], lhsT,
                                    t_a[:, j:j + 2, c0:c1], perf_mode=DR,
                                    start=(sp == 0), stop=(sp == n_sp - 1))
                        for ci, (c0, c1) in enumerate(CHUNKS):
                            nc.vector.tensor_scalar_add(
                                t_aggTh[half][:, c0:c1],
                                af2[ci][0:64, 0:c1 - c0],
                                t_b2h[:, half:half + 1])
                    for m in range(M_TILES):
                        pst = tr_slot(m)
                        for half in range(2):
                            nc.tensor.transpose(
                                pst[:, 64 * half:64 * half + 64],
                                t_aggTh[half][:, m * 128:(m + 1) * 128],
                                t_id[0:64, 0:64])
                        nc.vector.tensor_copy(t_aggN[:, m, :], pst)
                        nc.scalar.activation(t_sqs[:], pst, Square,
                                             accum_out=t_ssqN[:, m:m + 1])
                    inv_chain(lambda mr: t_ssqN[:, mr], slice(0, M_TILES),
                              with_l2l2=True)
                    for m in range(M_TILES):
                        out_m(m, t_aggN[:, m, :])

    if compile:
        nc.compile()
    return nc


def _prep_inputs(X, A, W1, b1, W2, b2, agg_mode=None):
    """Host-side sharding/layout prep. Returns in_maps for the 8 cores."""
    agg_mode = agg_mode or AGG_MODE
    f32 = np.float32

    # --- A_hat (source-major): Ab[j, i] = 1 iff edge j->i, unit diag ---
    Ab = np.zeros((NP, NP), dtype=E4M3)
    Ab[:N, :N] = (np.asarray(A) != 0)
    idx = np.arange(N)
    Ab[idx, idx] = 1.0

    # k-tile order: chunk-a tiles (per-core m 0:6) then chunk-b (m 6:10),
    # both core-major — matches the AllGather output layout.
    korder = ([c * M_TILES + m for c in range(N_CORES) for m in range(MA)]
              + [c * M_TILES + m for c in range(N_CORES)
                 for m in range(MA, M_TILES)])

    # --- X^T (bf16), padded ---
    Xp = np.zeros((NP, F), dtype=f32)
    Xp[:N] = np.asarray(X, dtype=f32)
    XT = np.ascontiguousarray(Xp.T).astype(BF16)      # [256, NP]

    w1_host = np.ascontiguousarray(
        np.asarray(W1, dtype=f32).reshape(2, 128, H)
        .transpose(1, 0, 2)).astype(BF16)              # [128, 2, H]
    w2_host = np.asarray(W2, dtype=f32).astype(BF16)   # [128, H]
    w2h_host = np.ascontiguousarray(
        (SC2 * np.asarray(W2, dtype=f32)).reshape(2, 64, H)
        .transpose(1, 0, 2)).astype(BF16)              # [64, 2, H], SC2-folded

    b1s = SC1 * np.asarray(b1, dtype=f32)
    b2s = SC2 * np.asarray(b2, dtype=f32)
    b1c = np.ascontiguousarray(b1s.reshape(128, 1))
    b2c = np.ascontiguousarray(b2s.reshape(128, 1))
    b1h = np.ascontiguousarray(b1s.reshape(2, 64).T)   # [64, 2]
    b2h = np.ascontiguousarray(b2s.reshape(2, 64).T)   # [64, 2]
    ones_host = np.ones((128, 1), dtype=BF16)
    ident = np.eye(128, dtype=f32)

    in_maps = []
    for c in range(N_CORES):
        cols = slice(c * PER_CORE, (c + 1) * PER_CORE)
        S = Ab[:, cols].reshape(K_TILES, 128, PER_CORE)
        a_pre_c = np.ascontiguousarray(S[korder])
        xt_c = np.ascontiguousarray(
            XT[:, cols].reshape(2, 128, PER_CORE).transpose(1, 0, 2))
        im = {
            "a_pre": a_pre_c,
            "xt": xt_c,
            "w1": w1_host,
            "ones": ones_host,
            "ident": ident,
        }
        if agg_mode == "drdr":
            im.update({"w2h": w2h_host, "b1h": b1h, "b2h": b2h})
        else:
            im.update({"w2": w2_host, "b1c": b1c, "b2c": b2c, "b2h": b2h})
        in_maps.append(im)
    return in_maps


def _get_nc(agg_mode=None):
    agg_mode = agg_mode or AGG_MODE
    key = f"nc_{agg_mode}"
    if key not in _CACHE:
        _CACHE[key] = _build_nc(agg_mode)
    return _CACHE[key]


def kernel(X, A, W1, b1, W2, b2, _trace=False, _trace_kwargs=None):
    nc = _get_nc()
    in_maps = _prep_inputs(X, A, W1, b1, W2, b2, AGG_MODE)
    kw = {}
    if _trace:
        kw.update(trace=True, **(_trace_kwargs or {}))
    res = bass_utils.run_bass_kernel_spmd(
        nc, in_maps, core_ids=list(range(N_CORES)), **kw)
    _CACHE["last_result"] = res
    out = np.concatenate([res.results[c]["out"] for c in range(N_CORES)],
                         axis=0)[:N]
    return np.ascontiguousarray(out.astype(np.float32))

